# revision 1
# baseline (speedup 1.0000x reference)
"""CrossSliceAttention2D Trainium2 kernel (8 NeuronCores, SPMD).

Problem: B=4, C=256, H=W=48 (N=2304 pixels), 8 heads x head_dim 48.
  q = conv1x1(GN(q_feat)); k = conv1x1(kv_feat); v = conv1x1(kv_feat)
  out = conv1x1(softmax(q k^T / sqrt(48)) v) + bo + q_feat

Sharding: core (b, j) = batch b, query-pixel half j (1152 pixels).
Each core computes all 8 heads for its query rows against all 2304 kv
pixels, plus the full output projection for those rows -> outputs are
disjoint, no collectives; host just concatenates.

Device-side layout tricks:
  * All matmuls in bf16 (fp32 matmul is 4 cyc/row on PE, bf16 is 1).
  * K/Q kept in "head pair" layout: heads 2g / 2g+1 on partitions
    0-47 / 64-111 of tile g, so a head's 48 contraction rows never
    cross a 128-partition boundary.
  * Scores computed transposed (kv pixel on partitions, q on free dim)
    so exp'd tiles feed the AV matmul as the moving operand directly.
  * softmax: no max subtraction (scores are ~N(0, 0.1); |s| < 1), row
    sums via an all-ones 49th column on V^T, division applied to the
    [49 x q] AV output (tiny), broadcast of 1/rowsum across partitions
    done with a K=1 matmul on the PE.
  * GroupNorm stats via bn_stats/bn_aggr, group-combine and
    channel-broadcast via tiny indicator matmuls.
"""

import numpy as np

import concourse.bass as bass
import concourse.mybir as mybir
import concourse.tile as tile
from concourse import bacc
from concourse.bass_utils import run_bass_kernel_spmd

F32 = mybir.dt.float32
F32R = mybir.dt.float32r
BF16 = mybir.dt.bfloat16
AF = mybir.ActivationFunctionType
OP = mybir.AluOpType

P = 128
B = 4
C = 256          # io channels
NPIX = 2304      # 48*48 kv pixels
QH = NPIX // 2   # query pixels per core
HEADS = 8
D = 48           # head dim
INNER = 384
GROUPS = 32
EPS = 1e-5
SCALE = D ** -0.5
KT = NPIX // P   # 18 kv-pixel tiles

VW = 65  # V block width per head: cols 0-47 = V, 48-63 = 0, 64 = ones
Q_CHUNKS = [(0, 512), (512, 512), (1024, 128)]
N_CHUNKS = [(0, 512), (512, 512), (1024, 512), (1536, 512), (2048, 256)]
# double-kt QK psum [128, 2304]: kt even at cols 0-1151, kt odd at 1152-2303;
# matmul chunks may not cross 512-aligned PSUM bank boundaries:
DQ_A = [(0, 512), (512, 512), (1024, 128)]
DQ_B = [(1152, 384), (1536, 512), (2048, 256)]
# merged projection drain chunks
NK_CHUNKS = [(0, 1024), (1024, 1024), (2048, 256)]
QK_CHUNKS = [(0, 1024), (1024, 128)]
N_OFF = 0  # double-kt exp tiles per head offloaded to the Pool engine


def _build(stage="full", loops=1):
    nc = bacc.Bacc("TRN2", debug=False, target_bir_lowering=False, num_devices=8)

    xq_d = nc.dram_tensor("xq", [C, NPIX], F32, kind="ExternalInput").ap()
    xkv_d = nc.dram_tensor("xkv", [C, NPIX], F32, kind="ExternalInput").ap()
    # wqT/wkT in padded "pair" column layout: head h at cols
    # 128*(h//2) + 64*(h%2), cols 48-63 / 112-127 of each block zero.
    wq_d = nc.dram_tensor("wqT", [C, 4 * P], F32, kind="ExternalInput").ap()
    wk_d = nc.dram_tensor("wkT", [C, 4 * P], F32, kind="ExternalInput").ap()
    wv_d = nc.dram_tensor("wvT", [C, INNER], F32, kind="ExternalInput").ap()
    # woT in "pair" row layout: head h lives at rows 128*(h//2) + 64*(h%2),
    # rows 48-63 / 112-127 of each 128-block are zero.
    wo_d = nc.dram_tensor("woT", [4 * P, C], F32, kind="ExternalInput").ap()
    bqp_d = nc.dram_tensor("bqp", [P, 4], F32, kind="ExternalInput").ap()
    bkp_d = nc.dram_tensor("bkp", [P, 4], F32, kind="ExternalInput").ap()
    bv_d = nc.dram_tensor("bv", [1, INNER], F32, kind="ExternalInput").ap()
    bop_d = nc.dram_tensor("bop", [P, 2], F32, kind="ExternalInput").ap()
    gnw_d = nc.dram_tensor("gnwp", [P, 2], F32, kind="ExternalInput").ap()
    gnb_d = nc.dram_tensor("gnbp", [P, 2], F32, kind="ExternalInput").ap()
    gsum_d = nc.dram_tensor("gsum", [P, 2, GROUPS], F32, kind="ExternalInput").ap()
    gbc_d = nc.dram_tensor("gbc", [GROUPS, C], F32, kind="ExternalInput").ap()
    out_d = nc.dram_tensor("out", [C, QH], F32, kind="ExternalOutput").ap()

    with tile.TileContext(nc) as tc:
        for _it in range(loops):
            with (
                tc.tile_pool(name="persist", bufs=1) as persist,
                tc.tile_pool(name="tmp", bufs=3) as tmp,
            ):
                # ---------------- persistent tiles + input DMA ----------------
                xq_sb = persist.tile([P, 2, NPIX], F32, tag="xq")
                xq_r = xq_d.rearrange("(t p) n -> p t n", p=P)
                for t in range(2):
                    nc.sync.dma_start(out=xq_sb[:, t], in_=xq_r[:, t])

                bqp = persist.tile([P, 4], F32, tag="bqp")
                nc.sync.dma_start(out=bqp, in_=bqp_d)
                bkp = persist.tile([P, 4], F32, tag="bkp")
                nc.sync.dma_start(out=bkp, in_=bkp_d)
                bop = persist.tile([P, 2], F32, tag="bop")
                nc.sync.dma_start(out=bop, in_=bop_d)
                gnw = persist.tile([P, 2], F32, tag="gnw")
                nc.sync.dma_start(out=gnw, in_=gnw_d)
                gnb = persist.tile([P, 2], F32, tag="gnb")
                nc.sync.dma_start(out=gnb, in_=gnb_d)
                gsum = persist.tile([P, 2, GROUPS], F32, tag="gsum")
                nc.sync.dma_start(out=gsum, in_=gsum_d)
                gbc = persist.tile([GROUPS, C], F32, tag="gbc")
                nc.sync.dma_start(out=gbc, in_=gbc_d)

                ones_row = persist.tile([1, P], BF16, tag="ones_row")
                nc.vector.memset(ones_row, 1.0)
                ones_f32 = persist.tile([1, D], F32, tag="ones_f32")
                nc.vector.memset(ones_f32, 1.0)
                ones_f32r = persist.tile([1, D], F32R, tag="ones_f32r")
                with nc.allow_low_precision(reason="exact 1.0 cast to f32r"):
                    nc.vector.tensor_copy(out=ones_f32r, in_=ones_f32)
                zeros_col = persist.tile([P, 1], F32, tag="zeros_col")
                nc.vector.memset(zeros_col, 0.0)
                eps_col = persist.tile([P, 1], F32, tag="eps_col")
                nc.vector.memset(eps_col, EPS)

                kpair = persist.tile([P, 4, NPIX], BF16, tag="kpair")
                qpair = persist.tile([P, 4, QH], BF16, tag="qpair")
                vT = persist.tile([P, KT, HEADS * VW], BF16, tag="vt")
                # zero the 48..63 pad cols, ones in col 64 of each head block
                vT4 = vT.rearrange("p t (h c) -> p t h c", c=VW)
                nc.gpsimd.memset(vT4[:, :, :, D : VW - 1], 0.0)
                nc.gpsimd.memset(vT4[:, :, :, VW - 1 : VW], 1.0)
                # o in pair layout (like K/Q); pad rows stay zero
                o_pad = persist.tile([P, 4, QH], BF16, tag="opad")
                nc.gpsimd.memset(o_pad, 0.0)

                with (
                    tc.tile_pool(name="stage", bufs=1) as stg,
                    tc.tile_pool(name="ps1", bufs=4, space="PSUM") as ps1,
                ):
                    # ------------- load + cast weights to bf16 -------------
                    def load_w(dram_ap, name):
                        k, f = dram_ap.shape
                        t = k // P
                        w32 = stg.tile([P, t, f], F32, tag=f"{name}32")
                        nc.gpsimd.dma_start(
                            out=w32, in_=dram_ap.rearrange("(t p) f -> p t f", p=P)
                        )
                        wbf = persist.tile([P, t, f], BF16, tag=name)
                        nc.gpsimd.tensor_copy(out=wbf, in_=w32)
                        return wbf

                    wq_bf = load_w(wq_d, "wq")
                    wk_bf = load_w(wk_d, "wk")
                    wv_bf = load_w(wv_d, "wv")
                    wo_bf = load_w(wo_d, "wo")

                    bv32 = stg.tile([1, INNER], F32, tag="bv32")
                    nc.gpsimd.dma_start(out=bv32, in_=bv_d)
                    bv_bf = persist.tile([1, INNER], BF16, tag="bv")
                    nc.gpsimd.tensor_copy(out=bv_bf, in_=bv32)

                    xkv32 = stg.tile([P, 2, NPIX], F32, tag="xkv32")
                    xkv_bf = stg.tile([P, 2, NPIX], BF16, tag="xkvbf")
                    xkv_r = xkv_d.rearrange("(t p) n -> p t n", p=P)
                    for t in range(2):
                        nc.scalar.dma_start(out=xkv32[:, t], in_=xkv_r[:, t])
                        nc.gpsimd.tensor_copy(out=xkv_bf[:, t], in_=xkv32[:, t])

                    # ------------- GroupNorm stats on xq -------------
                    # per-channel mean/var, then 8-channel group combine via
                    # indicator matmul, then broadcast back to channels.
                    grp = persist.tile([GROUPS, 2], F32, tag="grp")
                    SUB = 9  # 2304 = 9 * 256 subgroups for bn_stats
                    ps_stat = ps1.tile([P, 512], F32, tag="p")
                    for t in range(2):
                        st = stg.tile([P, SUB, 6], F32, tag=f"bnst{t}")
                        xr = xq_sb[:, t].rearrange("p (s f) -> p s f", s=SUB)
                        for s in range(SUB):
                            nc.vector.bn_stats(out=st[:, s], in_=xr[:, s])
                        mv = stg.tile([P, 2], F32, tag=f"mv{t}")
                        nc.vector.bn_aggr(out=mv, in_=st)
                        # mv[:,1] (var) += mean^2  -> E[x^2]
                        msq = tmp.tile([P, 1], F32, tag="msq")
                        nc.vector.tensor_mul(out=msq, in0=mv[:, 0:1], in1=mv[:, 0:1])
                        nc.vector.tensor_add(out=mv[:, 1:2], in0=mv[:, 1:2], in1=msq)
                        # accumulate both channel-tiles into one [32, 2] psum
                        nc.tensor.matmul(
                            ps_stat[0:GROUPS, 0:2], gsum[:, t], mv,
                            start=(t == 0), stop=(t == 1),
                        )
                    nc.vector.tensor_copy(out=grp, in_=ps_stat[0:GROUPS, 0:2])
                    # group var = E[x^2] - mu^2 ; rstd = 1/sqrt(var + eps)
                    msq2 = tmp.tile([GROUPS, 1], F32, tag="msq32")
                    nc.vector.tensor_mul(out=msq2, in0=grp[:, 0:1], in1=grp[:, 0:1])
                    nc.vector.tensor_tensor(
                        out=grp[:, 1:2], in0=grp[:, 1:2], in1=msq2, op=OP.subtract
                    )
                    nc.scalar.activation(
                        out=grp[:, 1:2], in_=grp[:, 1:2], func=AF.Sqrt,
                        bias=eps_col[:GROUPS],
                    )
                    nc.vector.reciprocal(out=grp[:, 1:2], in_=grp[:, 1:2])

                    # per-channel affine: gn(x) = A*x + Cc
                    AC = persist.tile([P, 2, 2], F32, tag="ac")
                    gnq = stg.tile([P, 2, NPIX], BF16, tag="gnq")
                    for t in range(2):
                        ps = ps1.tile([P, 512], F32, tag="p")
                        nc.tensor.matmul(
                            ps[:, 0:2],
                            gbc[:, t * P : (t + 1) * P],
                            grp,
                            start=True,
                            stop=True,
                        )
                        # A = gnw * rstd_bcast
                        nc.vector.tensor_mul(
                            out=AC[:, t, 0:1], in0=gnw[:, t : t + 1], in1=ps[:, 1:2]
                        )
                        # Cc = gnb - mu_bcast * A
                        mt_ = tmp.tile([P, 1], F32, tag="msq")
                        nc.vector.tensor_mul(out=mt_, in0=ps[:, 0:1], in1=AC[:, t, 0:1])
                        nc.vector.tensor_tensor(
                            out=AC[:, t, 1:2], in0=gnb[:, t : t + 1], in1=mt_,
                            op=OP.subtract,
                        )
                        nc.vector.tensor_scalar(
                            out=gnq[:, t],
                            in0=xq_sb[:, t],
                            scalar1=AC[:, t, 0:1],
                            scalar2=AC[:, t, 1:2],
                            op0=OP.mult,
                            op1=OP.add,
                        )

                    # ------------- K / Q projections (head-pair layout) -------------
                    # pair g holds head 2g on partitions 0-47, head 2g+1 on 64-111
                    def proj_pair(g, w_bf, rhs, chunks, dst, bias):
                        for (o, w) in chunks:
                            ps = ps1.tile([P, 1024], F32, tag="p")
                            for so in range(0, w, 512):
                                sw = min(512, w - so)
                                for kp in range(2):
                                    nc.tensor.matmul(
                                        ps[:, so : so + sw],
                                        w_bf[:, kp, g * P : (g + 1) * P],
                                        rhs[:, kp, o + so : o + so + sw],
                                        start=(kp == 0),
                                        stop=(kp == 1),
                                    )
                            nc.vector.tensor_scalar_add(
                                out=dst[:, g, o : o + w],
                                in0=ps[:, 0:w],
                                scalar1=bias[:, g : g + 1],
                            )

                    for g in range(4):
                        proj_pair(g, wk_bf, xkv_bf, NK_CHUNKS, kpair, bkp)
                        proj_pair(g, wq_bf, gnq, QK_CHUNKS, qpair, bqp)

                    # ------------- V^T projection (kv pixel major) -------------
                    # ones in the 49th column of each head block (row sums)
                    nc.vector.memset(
                        vT.rearrange("p t (h c) -> p t h c", c=VW)[:, :, :, VW - 1 : VW],
                        1.0,
                    )
                    for pt in range(KT):
                        ps = ps1.tile([P, 512], F32, tag="p")
                        for kp in range(2):
                            nc.tensor.matmul(
                                ps[:, 0:INNER],
                                xkv_bf[:, kp, pt * P : (pt + 1) * P],
                                wv_bf[:, kp],
                                start=(kp == 0),
                                stop=False,
                            )
                        # bias via K=1 matmul: += ones^T @ bv
                        nc.tensor.matmul(
                            ps[:, 0:INNER],
                            ones_row,
                            bv_bf,
                            start=False,
                            stop=True,
                        )
                        nc.vector.tensor_copy(
                            out=vT[:, pt].rearrange("p (h c) -> p h c", c=VW)[:, :, 0:D],
                            in_=ps[:, 0:INNER].rearrange("p (h c) -> p h c", c=D),
                        )

                def _dump(src0, src1):
                    with tc.tile_pool(name="dbg", bufs=2) as dbg:
                        for mt, src in ((0, src0), (1, src1)):
                            t = dbg.tile([P, QH], F32, tag="dbg")
                            nc.vector.tensor_copy(out=t, in_=src)
                            nc.sync.dma_start(
                                out=out_d[mt * P : (mt + 1) * P, :], in_=t
                            )

                if stage == "proj":
                    _dump(kpair[:, 0, 0:QH], qpair[:, 0, :])

                # ---------------- attention ----------------
                n_heads = 0 if stage == "proj" else {"qk": 1, "av": 1, "av2": 2}.get(stage, HEADS)
                with (
                    tc.tile_pool(name="attn", bufs=18) as attn_pool,
                    tc.tile_pool(name="rdram", bufs=4, space="DRAM") as rdram,
                    tc.tile_pool(name="psqk", bufs=1, space="PSUM") as psqk,
                    tc.tile_pool(name="psav", bufs=3, space="PSUM") as psav,
                ):
                    for h in range(n_heads):
                        g, half = divmod(h, 2)
                        po = 64 * half
                        atiles = []
                        for kd in range(KT // 2):
                            # two kv-pixel tiles share one [128, 2304] psum + one exp
                            ps = psqk.tile([P, 2 * QH], F32, tag="qk")
                            for kth, chunks in ((0, DQ_A), (1, DQ_B)):
                                kt = 2 * kd + kth
                                qbase = QH * kth
                                for (o, w) in chunks:
                                    nc.tensor.matmul(
                                        ps[:, o : o + w],
                                        kpair[po : po + 48, g, kt * P : (kt + 1) * P],
                                        qpair[po : po + 48, g, o - qbase : o - qbase + w],
                                        start=True,
                                        stop=True,
                                    )
                            at = attn_pool.tile([P, 2 * QH], BF16, tag="attn")
                            if kd >= KT // 2 - N_OFF:
                                # Pool-engine polynomial exp offload:
                                # exp(s*SCALE) = p(u)^2, u = s*SCALE/2,
                                # p(u) = 1 + u + u^2/2 + u^3/6  (|u| < ~0.4)
                                u = tmp.tile([P, 2 * QH], BF16, tag="pu")
                                nc.vector.tensor_scalar_mul(
                                    out=u, in0=ps, scalar1=SCALE / 2.0
                                )
                                t1 = tmp.tile([P, 2 * QH], BF16, tag="pt")
                                nc.gpsimd.tensor_scalar(
                                    out=t1, in0=u, scalar1=1.0 / 6.0, scalar2=0.5,
                                    op0=OP.mult, op1=OP.add,
                                )
                                nc.gpsimd.tensor_tensor(out=t1, in0=t1, in1=u, op=OP.mult)
                                nc.gpsimd.tensor_scalar_add(out=t1, in0=t1, scalar1=1.0)
                                nc.gpsimd.tensor_tensor(out=t1, in0=t1, in1=u, op=OP.mult)
                                nc.gpsimd.tensor_scalar_add(out=t1, in0=t1, scalar1=1.0)
                                nc.gpsimd.tensor_tensor(out=at, in0=t1, in1=t1, op=OP.mult)
                            else:
                                nc.scalar.activation(
                                    out=at, in_=ps, func=AF.Exp, scale=SCALE,
                                    bias=zeros_col,
                                )
                            atiles.append(at)

                        if stage == "qk":
                            _dump(atiles[0][:, 0:QH], atiles[0][:, QH : 2 * QH])
                            continue

                        # AV for all three q-chunks; collect 1/rowsum rows, then
                        # one DRAM-round-trip partition broadcast per head
                        # (DRAM sources allow partition-step-0 APs).
                        pavs = []
                        rc = tmp.tile([1, QH], BF16, tag="rc")
                        for (o, w) in Q_CHUNKS:
                            pav = psav.tile([P, 512], F32, tag="av")
                            for kt in range(KT):
                                nc.tensor.matmul(
                                    pav[0:VW, 0:w],
                                    vT[:, kt, VW * h : VW * (h + 1)],
                                    atiles[kt // 2][:, QH * (kt % 2) + o : QH * (kt % 2) + o + w],
                                    start=(kt == 0),
                                    stop=(kt == KT - 1),
                                )
                            with nc.allow_low_precision(
                                reason="softmax 1/rowsum in bf16; ~4e-3 is fine"
                            ):
                                nc.vector.reciprocal(
                                    out=rc[:, o : o + w], in_=pav[VW - 1 : VW, 0:w]
                                )
                            pavs.append(pav)
                        rdr = rdram.tile([1, QH], BF16, tag="rdr")
                        nc.sync.dma_start(out=rdr, in_=rc)
                        rcs = tmp.tile([D, QH], BF16, tag="rcs")
                        row_bc = bass.AP(
                            tensor=rdr.tensor,
                            offset=rdr.offset,
                            ap=[[0, D]] + list(rdr[0:1, :].ap[1:]),
                        )
                        nc.sync.dma_start(out=rcs, in_=row_bc)
                        for (o, w), pav in zip(Q_CHUNKS, pavs):
                            nc.vector.tensor_tensor(
                                out=o_pad[po : po + D, g, o : o + w],
                                in0=pav[0:D, 0:w],
                                in1=rcs[:, o : o + w],
                                op=OP.mult,
                            )

                    if stage in ("av", "av2", "heads"):
                        _dump(o_pad[:, 0, :], o_pad[:, 0, :])

                    # ---------------- output projection + residual ----------------
                    for mt in range(2 if stage == "full" else 0):
                        for (o, w) in Q_CHUNKS:
                            ps = psav.tile([P, 512], F32, tag="av")
                            for kp in range(4):
                                nc.tensor.matmul(
                                    ps[:, 0:w],
                                    wo_bf[:, kp, mt * P : (mt + 1) * P],
                                    o_pad[:, kp, o : o + w],
                                    start=(kp == 0),
                                    stop=(kp == 3),
                                )
                            osb = tmp.tile([P, 512], F32, tag="osb")
                            nc.vector.tensor_scalar_add(
                                out=osb[:, 0:w], in0=ps[:, 0:w],
                                scalar1=bop[:, mt : mt + 1],
                            )
                            nc.vector.tensor_tensor(
                                out=osb[:, 0:w],
                                in0=osb[:, 0:w],
                                in1=xq_sb[:, mt, o : o + w],
                                op=OP.add,
                            )
                            nc.sync.dma_start(
                                out=out_d[mt * P : (mt + 1) * P, o : o + w],
                                in_=osb[:, 0:w],
                            )
    nc.finalize()
    return nc


_CACHE = {}


def _get_nc():
    if "nc" not in _CACHE:
        _CACHE["nc"] = _build()
    return _CACHE["nc"]


def _host_consts():
    if "consts" in _CACHE:
        return _CACHE["consts"]
    gsum = np.zeros((P, 2, GROUPS), np.float32)
    for t in range(2):
        for p in range(P):
            gsum[p, t, 16 * t + p // 8] = 1.0 / 8.0
    gbc = np.zeros((GROUPS, C), np.float32)
    for c in range(C):
        gbc[c // 8, c] = 1.0
    _CACHE["consts"] = (gsum, gbc)
    return _CACHE["consts"]


def _pair_wo(woT):
    # [384, 256] -> [512, 256]; head h rows at 128*(h//2) + 64*(h%2)
    out = np.zeros((4 * P, C), np.float32)
    for g in range(4):
        for half in range(2):
            out[P * g + 64 * half : P * g + 64 * half + D] = woT[
                96 * g + D * half : 96 * g + D * half + D
            ]
    return out


def _pair_wT(wT):
    # [256, 384] -> [256, 512]; head h cols at 128*(h//2) + 64*(h%2)
    out = np.zeros((C, 4 * P), np.float32)
    for g in range(4):
        for half in range(2):
            out[:, P * g + 64 * half : P * g + 64 * half + D] = wT[
                :, 96 * g + D * half : 96 * g + D * half + D
            ]
    return out


def _pair_bias(b):
    out = np.zeros((P, 4), np.float32)
    for g in range(4):
        out[0:48, g] = b[96 * g : 96 * g + 48]
        out[64:112, g] = b[96 * g + 48 : 96 * g + 96]
    return out


def _split_bias(b):
    # [2k*128] -> [128, 2k] partition-major
    n = b.shape[0] // P
    return np.ascontiguousarray(b.reshape(n, P).T)


def run(inputs, **kwargs):
    q_feat = np.asarray(inputs["q_feat"], np.float32).reshape(B, C, NPIX)
    kv_feat = np.asarray(inputs["kv_feat"], np.float32).reshape(B, C, NPIX)
    wqT = _pair_wT(np.ascontiguousarray(np.asarray(inputs["wq"], np.float32).T))
    wkT = _pair_wT(np.ascontiguousarray(np.asarray(inputs["wk"], np.float32).T))
    wvT = np.ascontiguousarray(np.asarray(inputs["wv"], np.float32).T)
    woT = _pair_wo(np.ascontiguousarray(np.asarray(inputs["wo"], np.float32).T))
    bqp = _pair_bias(np.asarray(inputs["bq"], np.float32))
    bkp = _pair_bias(np.asarray(inputs["bk"], np.float32))
    bv = np.asarray(inputs["bv"], np.float32).reshape(1, INNER)
    bop = _split_bias(np.asarray(inputs["bo"], np.float32))
    gnwp = _split_bias(np.asarray(inputs["gn_w"], np.float32))
    gnbp = _split_bias(np.asarray(inputs["gn_b"], np.float32))
    gsum, gbc = _host_consts()

    in_maps = []
    for b in range(B):
        for j in range(2):
            # roll so this core's query pixels land at columns 0..QH-1;
            # GroupNorm stats are permutation-invariant, kv side unaffected
            in_maps.append(
                {
                    "xq": np.ascontiguousarray(np.roll(q_feat[b], -QH * j, axis=1)),
                    "xkv": np.ascontiguousarray(kv_feat[b]),
                    "wqT": wqT,
                    "wkT": wkT,
                    "wvT": wvT,
                    "woT": woT,
                    "bqp": bqp,
                    "bkp": bkp,
                    "bv": bv,
                    "bop": bop,
                    "gnwp": gnwp,
                    "gnbp": gnbp,
                    "gsum": gsum,
                    "gbc": gbc,
                }
            )

    res = run_bass_kernel_spmd(_get_nc(), in_maps, core_ids=list(range(8)), **kwargs)

    out = np.empty((B, C, NPIX), np.float32)
    for i, r in enumerate(res.results):
        b, j = divmod(i, 2)
        out[b, :, QH * j : QH * (j + 1)] = r["out"]
    return out.reshape(B, C, 48, 48), res


def kernel(**inputs):
    out, _ = run(inputs)
    return out



# revision 2
# speedup vs baseline: 1.0174x; 1.0174x over previous
"""CrossSliceAttention2D Trainium2 kernel (8 NeuronCores, SPMD).

Problem: B=4, C=256, H=W=48 (N=2304 pixels), 8 heads x head_dim 48.
  q = conv1x1(GN(q_feat)); k = conv1x1(kv_feat); v = conv1x1(kv_feat)
  out = conv1x1(softmax(q k^T / sqrt(48)) v) + bo + q_feat

Sharding: core (b, j) = batch b, query-pixel half j (1152 pixels), all
heads; outputs disjoint, no collectives.

v2 design (vs baseline):
  * Head-PAIR concurrency on the PE: heads 2g/2g+1 live at partitions
    0-47 / 64-111, so their QK matmuls run in disjoint 32-row groups
    (tile_position row tiling) and their AV matmuls in disjoint col
    groups -> both heads stream simultaneously, ~2x PE throughput.
  * QK scores psum tile [128, 1024]: head A at cols 0-511 (bank k),
    head B at cols 512-1023 (bank k+1) -> concurrent matmuls hit
    different banks; ONE exp instruction covers both heads.
  * exp split between ACT (native Exp) and DVE (one-pass bit-trick:
    bf16 bits of exp(s*SCALE) = round(s*A_EXP + B_EXP), computed by
    tensor_scalar f32->uint16 aliased into the bf16 at-tile).
  * softmax 1/rowsum via bf16 bit-trick reciprocal (bits(1/x) =
    MAGIC16 - bits(x)) on the two rowsum rows, DMA-broadcast, one
    bf16 2x tensor_tensor multiply per pair. (Baseline: 60us of
    single-lane RECIPROCAL.)
  * bk dropped entirely (softmax shift invariance, exact); bv folded
    into bo on the host (rows of softmax sum to 1, exact).
  * AV pav [128, qw]: A on partitions 0-63, B on 64-127 (col tiling)
    -> one psum bank for both heads; drain is a single [128, qw] copy
    straight into o_pad pair layout.
"""

import numpy as np

import concourse.bass as bass
import concourse.mybir as mybir
import concourse.tile as tile
from concourse import bacc
from concourse.bass_utils import run_bass_kernel_spmd

F32 = mybir.dt.float32
BF16 = mybir.dt.bfloat16
U16 = mybir.dt.uint16
AF = mybir.ActivationFunctionType
OP = mybir.AluOpType

P = 128
B = 4
C = 256          # io channels
NPIX = 2304      # 48*48 kv pixels
QH = NPIX // 2   # query pixels per core
HEADS = 8
D = 48           # head dim
INNER = 384
GROUPS = 32
EPS = 1e-5
SCALE = D ** -0.5
KT = NPIX // P   # 18 kv-pixel tiles
VW = 64          # V block per head: col 0 = ones (rowsum), 1-48 = V, 49-63 = 0
                 # (rowsums land at partitions 0 / 64 - engine APs need
                 #  32-aligned partition starts)

Q_CHUNKS = [(0, 512), (512, 512), (1024, 128)]
NK_CHUNKS = [(0, 1024), (1024, 1024), (2048, 256)]
QK_CHUNKS = [(0, 1024), (1024, 128)]

A_EXP = SCALE * np.log2(np.e) * 128.0     # bf16-bits exp slope
B_EXP = (127.0 - 0.0430) * 128.0          # bf16-bits exp offset
MAGIC16 = 32498.0                         # bf16-bits reciprocal magic

# kt -> engine for the exp pass (True = ACT). 5/9 on ACT.
ACT_KT = [kt % 9 in (0, 2, 4, 6, 8) for kt in range(KT)]


def _build():
    nc = bacc.Bacc("TRN2", debug=False, target_bir_lowering=False, num_devices=8)

    xq_d = nc.dram_tensor("xq", [C, NPIX], F32, kind="ExternalInput").ap()
    xkv_d = nc.dram_tensor("xkv", [C, NPIX], F32, kind="ExternalInput").ap()
    # wqT/wkT in padded "pair" column layout: head h at cols
    # 128*(h//2) + 64*(h%2), cols 48-63 / 112-127 of each block zero.
    wq_d = nc.dram_tensor("wqT", [C, 4 * P], F32, kind="ExternalInput").ap()
    wk_d = nc.dram_tensor("wkT", [C, 4 * P], F32, kind="ExternalInput").ap()
    wv_d = nc.dram_tensor("wvT", [C, INNER], F32, kind="ExternalInput").ap()
    # woT in "pair" row layout: head h rows at 128*(h//2) + 64*(h%2),
    # rows 48-63 / 112-127 of each 128-block zero.
    wo_d = nc.dram_tensor("woT", [4 * P, C], F32, kind="ExternalInput").ap()
    bqp_d = nc.dram_tensor("bqp", [P, 4], F32, kind="ExternalInput").ap()
    bop_d = nc.dram_tensor("bop", [P, 2], F32, kind="ExternalInput").ap()
    gnw_d = nc.dram_tensor("gnwp", [P, 2], F32, kind="ExternalInput").ap()
    gnb_d = nc.dram_tensor("gnbp", [P, 2], F32, kind="ExternalInput").ap()
    gsum_d = nc.dram_tensor("gsum", [P, 2, GROUPS], F32, kind="ExternalInput").ap()
    gbc_d = nc.dram_tensor("gbc", [GROUPS, C], F32, kind="ExternalInput").ap()
    out_d = nc.dram_tensor("out", [C, QH], F32, kind="ExternalOutput").ap()

    with tile.TileContext(nc) as tc:
        with (
            tc.tile_pool(name="persist", bufs=1) as persist,
            tc.tile_pool(name="tmp", bufs=3) as tmp,
        ):
            # ---------------- persistent tiles + input DMA ----------------
            # (weight DMAs go FIRST on sync so warmup matmuls start early;
            #  xq DMAs follow, then the small parameter tensors)
            xq_sb = persist.tile([P, 2, NPIX], F32, tag="xq")
            xq_r = xq_d.rearrange("(t p) n -> p t n", p=P)
            wqkq32 = persist.tile([P, 2, 2, 4 * P], F32, tag="wqk32")
            for wi, w_d in enumerate((wq_d, wk_d)):
                nc.sync.dma_start(
                    out=wqkq32[:, wi],
                    in_=w_d.rearrange("(t p) f -> p t f", p=P),
                )
            for t in range(2):
                nc.sync.dma_start(out=xq_sb[:, t], in_=xq_r[:, t])

            bqp = persist.tile([P, 4], F32, tag="bqp")
            nc.sync.dma_start(out=bqp, in_=bqp_d)
            bop = persist.tile([P, 2], F32, tag="bop")
            nc.sync.dma_start(out=bop, in_=bop_d)
            gnw = persist.tile([P, 2], F32, tag="gnw")
            nc.sync.dma_start(out=gnw, in_=gnw_d)
            gnb = persist.tile([P, 2], F32, tag="gnb")
            nc.sync.dma_start(out=gnb, in_=gnb_d)
            gsum = persist.tile([P, 2, GROUPS], F32, tag="gsum")
            nc.sync.dma_start(out=gsum, in_=gsum_d)
            gbc = persist.tile([GROUPS, C], F32, tag="gbc")
            nc.sync.dma_start(out=gbc, in_=gbc_d)

            zeros_col = persist.tile([P, 1], F32, tag="zeros_col")
            nc.vector.memset(zeros_col, 0.0)
            eps_col = persist.tile([P, 1], F32, tag="eps_col")
            nc.vector.memset(eps_col, EPS)

            kpair = persist.tile([P, 4, NPIX], BF16, tag="kpair")
            # zero-masked Q copies: qz[0] has head-B rows zeroed, qz[1] has
            # head-A rows zeroed. QK then runs as a full-K=128 matmul against
            # the full kpair block (the mask kills the other head's rows):
            # full-array matmuls keep the HAM clock gate at 2.4 GHz, which
            # row-tiled K=48 matmuls do not.
            qz = persist.tile([P, 2, 4, QH], BF16, tag="qz")
            nc.vector.memset(qz[:, 0], 0.0)
            nc.vector.memset(qz[:, 1], 0.0)
            vT = persist.tile([P, KT, HEADS, VW], BF16, tag="vt")
            # col 0 = ones (rowsum), cols 1-48 = V, 49-63 = 0
            nc.vector.memset(vT[:, :, :, 0:1], 1.0)
            nc.vector.memset(vT[:, :, :, 1 + D : VW], 0.0)
            o_pad = persist.tile([P, 4, QH], BF16, tag="opad")

            with (
                tc.tile_pool(name="stage", bufs=1) as stg,
                tc.tile_pool(name="ps1", bufs=4, space="PSUM") as ps1,
            ):
                # ------------- load + cast weights to bf16 -------------
                wq_bf = persist.tile([P, 2, 4 * P], BF16, tag="wq")
                nc.vector.tensor_copy(out=wq_bf, in_=wqkq32[:, 0])
                wk_bf = persist.tile([P, 2, 4 * P], BF16, tag="wk")
                nc.vector.tensor_copy(out=wk_bf, in_=wqkq32[:, 1])

                def load_w(dram_ap, name):
                    k, f = dram_ap.shape
                    t = k // P
                    w32 = stg.tile([P, t, f], F32, tag=f"{name}32")
                    nc.gpsimd.dma_start(
                        out=w32, in_=dram_ap.rearrange("(t p) f -> p t f", p=P)
                    )
                    wbf = persist.tile([P, t, f], BF16, tag=name)
                    nc.vector.tensor_copy(out=wbf, in_=w32)
                    return wbf

                wv_bf = load_w(wv_d, "wv")
                wo_bf = load_w(wo_d, "wo")

                # HAM warmup: dense full-array matmuls while input DMAs land
                ps_w = ps1.tile([P, 1024], F32, tag="p")
                for i in range(14):
                    nc.tensor.matmul(
                        ps_w[:, 0:512],
                        wq_bf[:, 0, 0:P],
                        wq_bf.rearrange("p t f -> p (t f)")[:, 0:512],
                        start=True, stop=True,
                    )

                xkv32 = stg.tile([P, 2, NPIX], F32, tag="xkv32")
                xkv_bf = stg.tile([P, 2, NPIX], BF16, tag="xkvbf")
                xkv_r = xkv_d.rearrange("(t p) n -> p t n", p=P)
                for t in range(2):
                    nc.scalar.dma_start(out=xkv32[:, t], in_=xkv_r[:, t])
                    nc.vector.tensor_copy(out=xkv_bf[:, t], in_=xkv32[:, t])

                # GN statistics (DVE side) as early as possible; the group
                # combine + affine run later so the PE queue isn't blocked
                mvs = []
                for t in range(2):
                    st = stg.tile([P, 9, 6], F32, tag=f"bnst{t}")
                    xr = xq_sb[:, t].rearrange("p (s f) -> p s f", s=9)
                    for s in range(9):
                        nc.vector.bn_stats(out=st[:, s], in_=xr[:, s])
                    mv = stg.tile([P, 2], F32, tag=f"mv{t}")
                    nc.vector.bn_aggr(out=mv, in_=st)
                    msq = tmp.tile([P, 1], F32, tag="msq")
                    nc.vector.tensor_mul(out=msq, in0=mv[:, 0:1], in1=mv[:, 0:1])
                    nc.vector.tensor_add(out=mv[:, 1:2], in0=mv[:, 1:2], in1=msq)
                    mvs.append(mv)

                # ------------- K / Q projections (head-pair layout) -------------
                def proj_pair(g, w_bf, rhs, chunks, dst, bias, drain_act):
                    for ci, (o, w) in enumerate(chunks):
                        ps = ps1.tile([P, 1024], F32, tag="p")
                        for so in range(0, w, 512):
                            sw = min(512, w - so)
                            for kp in range(2):
                                nc.tensor.matmul(
                                    ps[:, so : so + sw],
                                    w_bf[:, kp, g * P : (g + 1) * P],
                                    rhs[:, kp, o + so : o + so + sw],
                                    start=(kp == 0),
                                    stop=(kp == 1),
                                )
                        if bias is not None:
                            # Q: split into the two zero-masked copies;
                            # A-half on ACT (Identity+bias), B-half on DVE
                            nc.scalar.activation(
                                out=dst[0:D, 0, g, o : o + w], in_=ps[0:D, 0:w],
                                func=AF.Identity, bias=bias[0:D, g : g + 1],
                            )
                            nc.vector.tensor_scalar(
                                out=dst[64 : 64 + D, 1, g, o : o + w],
                                in0=ps[64 : 64 + D, 0:w],
                                scalar1=bias[64 : 64 + D, g : g + 1], scalar2=None,
                                op0=OP.add,
                            )
                        elif drain_act:
                            nc.scalar.copy(out=dst[:, g, o : o + w], in_=ps[:, 0:w])
                        else:
                            nc.vector.tensor_copy(
                                out=dst[:, g, o : o + w], in_=ps[:, 0:w]
                            )

                for g in range(4):
                    proj_pair(g, wk_bf, xkv_bf, NK_CHUNKS, kpair, None, True)

                # ------------- V^T projection (kv pixel major, no bias) -------------
                # (before Q proj: Q waits on GN stats; the in-order PE queue
                #  would stall V behind it)
                for pt in range(KT):
                    ps = ps1.tile([P, 1024], F32, tag="p")
                    for kp in range(2):
                        nc.tensor.matmul(
                            ps[:, 0:INNER],
                            xkv_bf[:, kp, pt * P : (pt + 1) * P],
                            wv_bf[:, kp],
                            start=(kp == 0),
                            stop=(kp == 1),
                        )
                    dst = vT[:, pt, :, 1 : 1 + D]
                    src = ps[:, 0:INNER].rearrange("p (h c) -> p h c", c=D)
                    if pt % 2 == 0:
                        nc.scalar.copy(out=dst, in_=src)
                    else:
                        nc.vector.tensor_copy(out=dst, in_=src)

                # ------------- GroupNorm group combine -------------
                # (matmuls emitted after K/V so the in-order PE queue isn't
                #  blocked waiting for the stats chain)
                grp = persist.tile([GROUPS, 2], F32, tag="grp")
                ps_stat = ps1.tile([P, 1024], F32, tag="p")
                for t in range(2):
                    nc.tensor.matmul(
                        ps_stat[0:GROUPS, 0:2], gsum[:, t], mvs[t],
                        start=(t == 0), stop=(t == 1),
                    )
                nc.vector.tensor_copy(out=grp, in_=ps_stat[0:GROUPS, 0:2])
                # group var = E[x^2] - mu^2 ; rstd = 1/sqrt(var + eps)
                msq2 = tmp.tile([GROUPS, 1], F32, tag="msq32")
                nc.vector.tensor_mul(out=msq2, in0=grp[:, 0:1], in1=grp[:, 0:1])
                nc.vector.tensor_tensor(
                    out=grp[:, 1:2], in0=grp[:, 1:2], in1=msq2, op=OP.subtract
                )
                nc.scalar.activation(
                    out=grp[:, 1:2], in_=grp[:, 1:2], func=AF.Sqrt,
                    bias=eps_col[:GROUPS],
                )
                nc.vector.reciprocal(out=grp[:, 1:2], in_=grp[:, 1:2])

                # per-channel affine: gn(x) = A*x + Cc
                AC = persist.tile([P, 2, 2], F32, tag="ac")
                gnq = stg.tile([P, 2, NPIX], BF16, tag="gnq")
                for t in range(2):
                    ps = ps1.tile([P, 1024], F32, tag="p")
                    nc.tensor.matmul(
                        ps[:, 0:2], gbc[:, t * P : (t + 1) * P], grp,
                        start=True, stop=True,
                    )
                    nc.vector.tensor_mul(
                        out=AC[:, t, 0:1], in0=gnw[:, t : t + 1], in1=ps[:, 1:2]
                    )
                    mt_ = tmp.tile([P, 1], F32, tag="msq")
                    nc.vector.tensor_mul(out=mt_, in0=ps[:, 0:1], in1=AC[:, t, 0:1])
                    nc.vector.tensor_tensor(
                        out=AC[:, t, 1:2], in0=gnb[:, t : t + 1], in1=mt_,
                        op=OP.subtract,
                    )
                    nc.gpsimd.tensor_scalar(
                        out=gnq[:, t], in0=xq_sb[:, t],
                        scalar1=AC[:, t, 0:1], scalar2=AC[:, t, 1:2],
                        op0=OP.mult, op1=OP.add,
                    )

                # fold bo into the residual: xq_sb += bo (per partition)
                for t in range(2):
                    nc.gpsimd.tensor_scalar(
                        out=xq_sb[:, t], in0=xq_sb[:, t],
                        scalar1=bop[:, t : t + 1], scalar2=None, op0=OP.add,
                    )

                for g in range(4):
                    proj_pair(g, wq_bf, gnq, QK_CHUNKS, qz, bqp, False)

            # ---------------- attention ----------------
            with (
                tc.tile_pool(name="attn", bufs=6) as atp,
                tc.tile_pool(name="rcb_p", bufs=2) as rcb_p,
                tc.tile_pool(name="rdram", bufs=2, space="DRAM") as rdram,
                tc.tile_pool(name="psqk", bufs=3, space="PSUM") as psqk,
                tc.tile_pool(name="psav", bufs=2, space="PSUM") as psav,
            ):
                # bridge the proj->attention transition so HAM stays warm
                warm = psqk.tile([P, 1024], F32, tag="qk")
                for i in range(6):
                    nc.tensor.matmul(
                        warm[:, 0:512],
                        kpair[:, 0, 0:P],
                        qz[:, 0, 0, 0:512],
                        start=True, stop=True,
                    )
                for g in range(4):
                    for qci, (qo, qw) in enumerate(Q_CHUNKS):
                        pav = psav.tile([P, 512], F32, tag="av")
                        # batch kv-tiles per psum tile so small q-chunks get
                        # one big exp instead of many tiny latency-bound ones
                        GRP = max(1, 512 // qw)
                        groups = [
                            list(range(s, min(s + GRP, KT)))
                            for s in range(0, KT, GRP)
                        ]
                        for gi, grp_kts in enumerate(groups):
                            qk = psqk.tile([P, 1024], F32, tag="qk")
                            for j, kt in enumerate(grp_kts):
                                if (qw == 512 and kt % 3 != 0) or (
                                    qw == 128 and j % 4 != 0
                                ):
                                    # row-tiled concurrent pair: half the
                                    # streamed columns; the surrounding
                                    # full-array matmuls keep HAM warm
                                    nc.tensor.matmul(
                                        qk[:, j * qw : j * qw + qw],
                                        kpair[0:D, g, kt * P : (kt + 1) * P],
                                        qz[0:D, 0, g, qo : qo + qw],
                                        start=True, stop=True,
                                    )
                                    nc.tensor.matmul(
                                        qk[:, 512 + j * qw : 512 + j * qw + qw],
                                        kpair[64 : 64 + D, g, kt * P : (kt + 1) * P],
                                        qz[64 : 64 + D, 1, g, qo : qo + qw],
                                        start=True, stop=True,
                                    )
                                    continue
                                # full-K=128 matmuls vs the whole kpair
                                # block; zero-masked q selects the head.
                                nc.tensor.matmul(
                                    qk[:, j * qw : (j + 1) * qw],
                                    kpair[:, g, kt * P : (kt + 1) * P],
                                    qz[:, 0, g, qo : qo + qw],
                                    start=True, stop=True,
                                )
                                nc.tensor.matmul(
                                    qk[:, 512 + j * qw : 512 + (j + 1) * qw],
                                    kpair[:, g, kt * P : (kt + 1) * P],
                                    qz[:, 1, g, qo : qo + qw],
                                    start=True, stop=True,
                                )
                            at = atp.tile([P, 1024], BF16, tag="at")
                            gw = len(grp_kts) * qw
                            qk_v = qk.rearrange("p (b c) -> p b c", b=2)[:, :, 0:gw]
                            at_v = at.rearrange("p (b c) -> p b c", b=2)[:, :, 0:gw]
                            if ACT_KT[gi]:
                                nc.scalar.activation(
                                    out=at_v, in_=qk_v, func=AF.Exp, scale=SCALE,
                                    bias=zeros_col,
                                )
                            else:
                                nc.vector.tensor_scalar(
                                    out=at_v.bitcast(U16), in0=qk_v,
                                    scalar1=A_EXP, scalar2=B_EXP,
                                    op0=OP.mult, op1=OP.add,
                                )
                            # AV accumulate; col-tiled pair shares one bank
                            for j, kt in enumerate(grp_kts):
                                nc.tensor.matmul(
                                    pav[0:VW, 0:qw],
                                    vT[:, kt, 2 * g],
                                    at[:, j * qw : (j + 1) * qw],
                                    start=(kt == 0), stop=(kt == KT - 1),
                                )
                                nc.tensor.matmul(
                                    pav[VW:P, 0:qw],
                                    vT[:, kt, 2 * g + 1],
                                    at[:, 512 + j * qw : 512 + (j + 1) * qw],
                                    start=(kt == 0), stop=(kt == KT - 1),
                                )
                        # drain unnormalized o (+rowsum rows 63/127) to o_pad
                        if (g * 3 + qci) % 2 == 0:
                            nc.scalar.copy(
                                out=o_pad[:, g, qo : qo + qw], in_=pav[:, 0:qw]
                            )
                        else:
                            nc.vector.tensor_copy(
                                out=o_pad[:, g, qo : qo + qw], in_=pav[:, 0:qw]
                            )

                    # normalize pair g: 1/rowsum bit-trick + broadcast + mult
                    # (rowsums at partitions 0 / 64 - 32-aligned for the DVE)
                    rc = tmp.tile([P, QH], BF16, tag="rc")
                    for row in (0, VW):
                        nc.vector.tensor_scalar(
                            out=rc[row : row + 1].bitcast(U16),
                            in0=o_pad[row : row + 1, g, :].bitcast(U16),
                            scalar1=-1.0, scalar2=MAGIC16,
                            op0=OP.mult, op1=OP.add,
                        )
                    rdr = rdram.tile([2, QH], BF16, tag="rdr")
                    for i, row in enumerate((0, VW)):
                        nc.sync.dma_start(
                            out=rdr[i : i + 1], in_=rc[row : row + 1]
                        )
                    rcb = rcb_p.tile([P, QH], BF16, tag="rcb")
                    for h in range(2):
                        rowap = rdr[h : h + 1, :]
                        src = bass.AP(
                            tensor=rowap.tensor, offset=rowap.offset,
                            ap=[[0, VW]] + list(rowap.ap[1:]),
                        )
                        nc.sync.dma_start(out=rcb[VW * h : VW * (h + 1), :], in_=src)
                    nc.gpsimd.tensor_tensor(
                        out=o_pad[:, g, :], in0=o_pad[:, g, :], in1=rcb,
                        op=OP.mult,
                    )

                # ---------------- output projection + residual ----------------
                for mt in range(2):
                    for (qo, qw) in Q_CHUNKS:
                        ps = psqk.tile([P, 1024], F32, tag="qk")
                        for kp in range(4):
                            nc.tensor.matmul(
                                ps[:, 0:qw],
                                wo_bf[:, kp, mt * P : (mt + 1) * P],
                                o_pad[:, kp, qo : qo + qw],
                                start=(kp == 0), stop=(kp == 3),
                            )
                        osb = tmp.tile([P, 512], F32, tag="osb")
                        nc.vector.tensor_tensor(
                            out=osb[:, 0:qw], in0=ps[:, 0:qw],
                            in1=xq_sb[:, mt, qo : qo + qw], op=OP.add,
                        )
                        nc.sync.dma_start(
                            out=out_d[mt * P : (mt + 1) * P, qo : qo + qw],
                            in_=osb[:, 0:qw],
                        )
    nc.finalize()
    return nc


_CACHE = {}


def _get_nc():
    if "nc" not in _CACHE:
        _CACHE["nc"] = _build()
    return _CACHE["nc"]


def _host_consts():
    if "consts" in _CACHE:
        return _CACHE["consts"]
    gsum = np.zeros((P, 2, GROUPS), np.float32)
    for t in range(2):
        for p in range(P):
            gsum[p, t, 16 * t + p // 8] = 1.0 / 8.0
    gbc = np.zeros((GROUPS, C), np.float32)
    for c in range(C):
        gbc[c // 8, c] = 1.0
    _CACHE["consts"] = (gsum, gbc)
    return _CACHE["consts"]


def _pair_wo(woT):
    # [384, 256] -> [512, 256]; head h rows at 128*(h//2) + 64*(h%2) + 1
    # (+1: row 0 of each half holds the rowsum from the ones-column of V)
    out = np.zeros((4 * P, C), np.float32)
    for g in range(4):
        for half in range(2):
            r0 = P * g + 64 * half + 1
            out[r0 : r0 + D] = woT[96 * g + D * half : 96 * g + D * half + D]
    return out


def _pair_wT(wT):
    # [256, 384] -> [256, 512]; head h cols at 128*(h//2) + 64*(h%2)
    out = np.zeros((C, 4 * P), np.float32)
    for g in range(4):
        for half in range(2):
            out[:, P * g + 64 * half : P * g + 64 * half + D] = wT[
                :, 96 * g + D * half : 96 * g + D * half + D
            ]
    return out


def _pair_bias(b):
    out = np.zeros((P, 4), np.float32)
    for g in range(4):
        out[0:48, g] = b[96 * g : 96 * g + 48]
        out[64:112, g] = b[96 * g + 48 : 96 * g + 96]
    return out


def _split_bias(b):
    # [2k*128] -> [128, 2k] partition-major
    n = b.shape[0] // P
    return np.ascontiguousarray(b.reshape(n, P).T)


def run(inputs, **kwargs):
    q_feat = np.asarray(inputs["q_feat"], np.float32).reshape(B, C, NPIX)
    kv_feat = np.asarray(inputs["kv_feat"], np.float32).reshape(B, C, NPIX)
    wqT = _pair_wT(np.ascontiguousarray(np.asarray(inputs["wq"], np.float32).T))
    wkT = _pair_wT(np.ascontiguousarray(np.asarray(inputs["wk"], np.float32).T))
    wvT = np.ascontiguousarray(np.asarray(inputs["wv"], np.float32).T)
    woT = _pair_wo(np.ascontiguousarray(np.asarray(inputs["wo"], np.float32).T))
    bqp = _pair_bias(np.asarray(inputs["bq"], np.float32))
    # fold bv through wo (softmax rows sum to 1) and bk away (shift inv.)
    bo_eff = np.asarray(inputs["bo"], np.float32) + (
        np.asarray(inputs["wo"], np.float32) @ np.asarray(inputs["bv"], np.float32)
    )
    bop = _split_bias(bo_eff)
    gnwp = _split_bias(np.asarray(inputs["gn_w"], np.float32))
    gnbp = _split_bias(np.asarray(inputs["gn_b"], np.float32))
    gsum, gbc = _host_consts()

    in_maps = []
    for b in range(B):
        for j in range(2):
            # roll so this core's query pixels land at columns 0..QH-1;
            # GroupNorm stats are permutation-invariant, kv side unaffected
            in_maps.append(
                {
                    "xq": np.ascontiguousarray(np.roll(q_feat[b], -QH * j, axis=1)),
                    "xkv": np.ascontiguousarray(kv_feat[b]),
                    "wqT": wqT,
                    "wkT": wkT,
                    "wvT": wvT,
                    "woT": woT,
                    "bqp": bqp,
                    "bop": bop,
                    "gnwp": gnwp,
                    "gnbp": gnbp,
                    "gsum": gsum,
                    "gbc": gbc,
                }
            )

    res = run_bass_kernel_spmd(_get_nc(), in_maps, core_ids=list(range(8)), **kwargs)

    out = np.empty((B, C, NPIX), np.float32)
    for i, r in enumerate(res.results):
        b, j = divmod(i, 2)
        out[b, :, QH * j : QH * (j + 1)] = r["out"]
    return out.reshape(B, C, 48, 48), res


def kernel(**inputs):
    out, _ = run(inputs)
    return out


# revision 3
# speedup vs baseline: 1.0977x; 1.0790x over previous
"""CrossSliceAttention2D Trainium2 kernel v2 (8 NeuronCores, SPMD).

Problem: B=4, C=256, H=W=48 (N=2304 pixels), 8 heads x head_dim 48.
  q = conv1x1(GN(q_feat)); k = conv1x1(kv_feat); v = conv1x1(kv_feat)
  out = conv1x1(softmax(q k^T / sqrt(48)) v) + bo + q_feat

Sharding: core (b, j) = batch b, query-pixel half j (1152 pixels), all
heads; outputs disjoint, no collectives.

v2 design (vs baseline):
  * Head-PAIR concurrency on the PE: heads 2g/2g+1 live at partitions
    0-47 / 64-111, so their QK matmuls run in disjoint 32-row groups
    (tile_position row tiling) and their AV matmuls in disjoint col
    groups -> both heads stream simultaneously, ~2x PE throughput.
  * QK scores psum tile [128, 1024]: head A at cols 0-511 (bank k),
    head B at cols 512-1023 (bank k+1) -> concurrent matmuls hit
    different banks; ONE exp instruction covers both heads.
  * exp split between ACT (native Exp) and DVE (one-pass bit-trick:
    bf16 bits of exp(s*SCALE) = round(s*A_EXP + B_EXP), computed by
    tensor_scalar f32->uint16 aliased into the bf16 at-tile).
  * softmax 1/rowsum via bf16 bit-trick reciprocal (bits(1/x) =
    MAGIC16 - bits(x)) on the two rowsum rows, DMA-broadcast, one
    bf16 2x tensor_tensor multiply per pair. (Baseline: 60us of
    single-lane RECIPROCAL.)
  * bk dropped entirely (softmax shift invariance, exact); bv folded
    into bo on the host (rows of softmax sum to 1, exact).
  * AV pav [128, qw]: A on partitions 0-63, B on 64-127 (col tiling)
    -> one psum bank for both heads; drain is a single [128, qw] copy
    straight into o_pad pair layout.
"""

import numpy as np

import concourse.bass as bass
import concourse.mybir as mybir
import concourse.tile as tile
from concourse import bacc
from concourse.bass_utils import run_bass_kernel_spmd

F32 = mybir.dt.float32
BF16 = mybir.dt.bfloat16
U16 = mybir.dt.uint16
AF = mybir.ActivationFunctionType
OP = mybir.AluOpType

P = 128
B = 4
C = 256          # io channels
NPIX = 2304      # 48*48 kv pixels
QH = NPIX // 2   # query pixels per core
HEADS = 8
D = 48           # head dim
INNER = 384
GROUPS = 32
EPS = 1e-5
SCALE = D ** -0.5
KT = NPIX // P   # 18 kv-pixel tiles
VW = 64          # V block per head: col 0 = ones (rowsum), 1-48 = V, 49-63 = 0
                 # (rowsums land at partitions 0 / 64 - engine APs need
                 #  32-aligned partition starts)

Q_CHUNKS = [(0, 512), (512, 512), (1024, 128)]
NK_CHUNKS = [(0, 1024), (1024, 1024), (2048, 256)]
QK_CHUNKS = [(0, 1024), (1024, 128)]

A_EXP = SCALE * np.log2(np.e) * 128.0     # bf16-bits exp slope
B_EXP = (127.0 - 0.0430) * 128.0          # bf16-bits exp offset
MAGIC16 = 32498.0                         # bf16-bits reciprocal magic

# kt -> engine for the exp pass (True = ACT). 5/9 on ACT.
ACT_KT = [kt % 9 in (0, 2, 4, 6, 8) for kt in range(KT)]


def _build():
    nc = bacc.Bacc("TRN2", debug=False, target_bir_lowering=False, num_devices=8)

    xq_d = nc.dram_tensor("xq", [C, NPIX], F32, kind="ExternalInput").ap()
    xkv_d = nc.dram_tensor("xkv", [C, NPIX], F32, kind="ExternalInput").ap()
    # wqT/wkT in padded "pair" column layout: head h at cols
    # 128*(h//2) + 64*(h%2), cols 48-63 / 112-127 of each block zero.
    wq_d = nc.dram_tensor("wqT", [C, 4 * P], F32, kind="ExternalInput").ap()
    wk_d = nc.dram_tensor("wkT", [C, 4 * P], F32, kind="ExternalInput").ap()
    wv_d = nc.dram_tensor("wvT", [C, INNER], F32, kind="ExternalInput").ap()
    # woT in "pair" row layout: head h rows at 128*(h//2) + 64*(h%2),
    # rows 48-63 / 112-127 of each 128-block zero.
    wo_d = nc.dram_tensor("woT", [4 * P, C], F32, kind="ExternalInput").ap()
    bqp_d = nc.dram_tensor("bqp", [P, 4], F32, kind="ExternalInput").ap()
    bop_d = nc.dram_tensor("bop", [P, 2], F32, kind="ExternalInput").ap()
    gnw_d = nc.dram_tensor("gnwp", [P, 2], F32, kind="ExternalInput").ap()
    gnb_d = nc.dram_tensor("gnbp", [P, 2], F32, kind="ExternalInput").ap()
    gsum_d = nc.dram_tensor("gsum", [P, 2, GROUPS], F32, kind="ExternalInput").ap()
    gbc_d = nc.dram_tensor("gbc", [GROUPS, C], F32, kind="ExternalInput").ap()
    out_d = nc.dram_tensor("out", [C, QH], F32, kind="ExternalOutput").ap()

    with tile.TileContext(nc) as tc:
        with (
            tc.tile_pool(name="persist", bufs=1) as persist,
            tc.tile_pool(name="tmp", bufs=3) as tmp,
        ):
            # ---------------- persistent tiles + input DMA ----------------
            # (weight DMAs go FIRST on sync so warmup matmuls start early;
            #  xq DMAs follow, then the small parameter tensors)
            xq_sb = persist.tile([P, 2, NPIX], F32, tag="xq")
            xq_r = xq_d.rearrange("(t p) n -> p t n", p=P)
            wqkq32 = persist.tile([P, 2, 2, 4 * P], F32, tag="wqk32")
            for wi, w_d in enumerate((wq_d, wk_d)):
                nc.sync.dma_start(
                    out=wqkq32[:, wi],
                    in_=w_d.rearrange("(t p) f -> p t f", p=P),
                )
            for t in range(2):
                nc.sync.dma_start(out=xq_sb[:, t], in_=xq_r[:, t])

            bqp = persist.tile([P, 4], F32, tag="bqp")
            nc.sync.dma_start(out=bqp, in_=bqp_d)
            bop = persist.tile([P, 2], F32, tag="bop")
            nc.sync.dma_start(out=bop, in_=bop_d)
            gnw = persist.tile([P, 2], F32, tag="gnw")
            nc.sync.dma_start(out=gnw, in_=gnw_d)
            gnb = persist.tile([P, 2], F32, tag="gnb")
            nc.sync.dma_start(out=gnb, in_=gnb_d)
            gsum = persist.tile([P, 2, GROUPS], F32, tag="gsum")
            nc.sync.dma_start(out=gsum, in_=gsum_d)
            gbc = persist.tile([GROUPS, C], F32, tag="gbc")
            nc.sync.dma_start(out=gbc, in_=gbc_d)

            zeros_col = persist.tile([P, 1], F32, tag="zeros_col")
            nc.vector.memset(zeros_col, 0.0)
            eps_col = persist.tile([P, 1], F32, tag="eps_col")
            nc.vector.memset(eps_col, EPS)

            kpair = persist.tile([P, 4, NPIX], BF16, tag="kpair")
            # zero-masked Q copies: qz[0] has head-B rows zeroed, qz[1] has
            # head-A rows zeroed. QK then runs as a full-K=128 matmul against
            # the full kpair block (the mask kills the other head's rows):
            # full-array matmuls keep the HAM clock gate at 2.4 GHz, which
            # row-tiled K=48 matmuls do not.
            qz = persist.tile([P, 2, 4, QH], BF16, tag="qz")
            nc.vector.memset(qz[:, 0], 0.0)
            nc.vector.memset(qz[:, 1], 0.0)
            vT = persist.tile([P, KT, HEADS, VW], BF16, tag="vt")
            # col 0 = ones (rowsum), cols 1-48 = V, 49-63 = 0
            nc.vector.memset(vT[:, :, :, 0:1], 1.0)
            nc.vector.memset(vT[:, :, :, 1 + D : VW], 0.0)
            o_pad = persist.tile([P, 4, QH], BF16, tag="opad")

            with (
                tc.tile_pool(name="stage", bufs=1) as stg,
                tc.tile_pool(name="ps1", bufs=4, space="PSUM") as ps1,
            ):
                # ------------- load + cast weights to bf16 -------------
                wq_bf = persist.tile([P, 2, 4 * P], BF16, tag="wq")
                nc.vector.tensor_copy(out=wq_bf, in_=wqkq32[:, 0])
                wk_bf = persist.tile([P, 2, 4 * P], BF16, tag="wk")
                nc.vector.tensor_copy(out=wk_bf, in_=wqkq32[:, 1])

                def load_w(dram_ap, name):
                    k, f = dram_ap.shape
                    t = k // P
                    w32 = stg.tile([P, t, f], F32, tag=f"{name}32")
                    nc.gpsimd.dma_start(
                        out=w32, in_=dram_ap.rearrange("(t p) f -> p t f", p=P)
                    )
                    wbf = persist.tile([P, t, f], BF16, tag=name)
                    nc.vector.tensor_copy(out=wbf, in_=w32)
                    return wbf

                wv_bf = load_w(wv_d, "wv")
                wo_bf = load_w(wo_d, "wo")

                # HAM warmup: dense full-array matmuls while input DMAs land
                ps_w = ps1.tile([P, 1024], F32, tag="p")
                for i in range(14):
                    nc.tensor.matmul(
                        ps_w[:, 0:512],
                        wq_bf[:, 0, 0:P],
                        wq_bf.rearrange("p t f -> p (t f)")[:, 0:512],
                        start=True, stop=True,
                    )

                xkv32 = stg.tile([P, 2, NPIX], F32, tag="xkv32")
                xkv_bf = stg.tile([P, 2, NPIX], BF16, tag="xkvbf")
                xkv_r = xkv_d.rearrange("(t p) n -> p t n", p=P)
                for t in range(2):
                    nc.scalar.dma_start(out=xkv32[:, t], in_=xkv_r[:, t])
                    nc.vector.tensor_copy(out=xkv_bf[:, t], in_=xkv32[:, t])

                # GN statistics (DVE side) as early as possible; the group
                # combine + affine run later so the PE queue isn't blocked
                mvs = []
                for t in range(2):
                    st = stg.tile([P, 9, 6], F32, tag=f"bnst{t}")
                    xr = xq_sb[:, t].rearrange("p (s f) -> p s f", s=9)
                    for s in range(9):
                        nc.vector.bn_stats(out=st[:, s], in_=xr[:, s])
                    mv = stg.tile([P, 2], F32, tag=f"mv{t}")
                    nc.vector.bn_aggr(out=mv, in_=st)
                    msq = tmp.tile([P, 1], F32, tag="msq")
                    nc.vector.tensor_mul(out=msq, in0=mv[:, 0:1], in1=mv[:, 0:1])
                    nc.vector.tensor_add(out=mv[:, 1:2], in0=mv[:, 1:2], in1=msq)
                    mvs.append(mv)

                # ------------- K / Q projections (head-pair layout) -------------
                def proj_pair(g, w_bf, rhs, chunks, dst, bias, drain_act):
                    for ci, (o, w) in enumerate(chunks):
                        ps = ps1.tile([P, 1024], F32, tag="p")
                        for so in range(0, w, 512):
                            sw = min(512, w - so)
                            for kp in range(2):
                                nc.tensor.matmul(
                                    ps[:, so : so + sw],
                                    w_bf[:, kp, g * P : (g + 1) * P],
                                    rhs[:, kp, o + so : o + so + sw],
                                    start=(kp == 0),
                                    stop=(kp == 1),
                                )
                        if bias is not None:
                            # Q: split into the two zero-masked copies;
                            # A-half on ACT (Identity+bias), B-half on DVE
                            nc.scalar.activation(
                                out=dst[0:D, 0, g, o : o + w], in_=ps[0:D, 0:w],
                                func=AF.Identity, bias=bias[0:D, g : g + 1],
                            )
                            nc.vector.tensor_scalar(
                                out=dst[64 : 64 + D, 1, g, o : o + w],
                                in0=ps[64 : 64 + D, 0:w],
                                scalar1=bias[64 : 64 + D, g : g + 1], scalar2=None,
                                op0=OP.add,
                            )
                        elif drain_act:
                            nc.scalar.copy(out=dst[:, g, o : o + w], in_=ps[:, 0:w])
                        else:
                            nc.vector.tensor_copy(
                                out=dst[:, g, o : o + w], in_=ps[:, 0:w]
                            )

                for g in range(4):
                    proj_pair(g, wk_bf, xkv_bf, NK_CHUNKS, kpair, None, True)

                # ------------- V^T projection (kv pixel major, no bias) -------------
                # (before Q proj: Q waits on GN stats; the in-order PE queue
                #  would stall V behind it)
                for pt in range(KT):
                    ps = ps1.tile([P, 1024], F32, tag="p")
                    for kp in range(2):
                        nc.tensor.matmul(
                            ps[:, 0:INNER],
                            xkv_bf[:, kp, pt * P : (pt + 1) * P],
                            wv_bf[:, kp],
                            start=(kp == 0),
                            stop=(kp == 1),
                        )
                    dst = vT[:, pt, :, 1 : 1 + D]
                    src = ps[:, 0:INNER].rearrange("p (h c) -> p h c", c=D)
                    if pt % 2 == 0:
                        nc.scalar.copy(out=dst, in_=src)
                    else:
                        nc.vector.tensor_copy(out=dst, in_=src)

                # ------------- GroupNorm group combine -------------
                # (matmuls emitted after K/V so the in-order PE queue isn't
                #  blocked waiting for the stats chain)
                grp = persist.tile([GROUPS, 2], F32, tag="grp")
                ps_stat = ps1.tile([P, 1024], F32, tag="p")
                for t in range(2):
                    nc.tensor.matmul(
                        ps_stat[0:GROUPS, 0:2], gsum[:, t], mvs[t],
                        start=(t == 0), stop=(t == 1),
                    )
                nc.vector.tensor_copy(out=grp, in_=ps_stat[0:GROUPS, 0:2])
                # group var = E[x^2] - mu^2 ; rstd = 1/sqrt(var + eps)
                msq2 = tmp.tile([GROUPS, 1], F32, tag="msq32")
                nc.vector.tensor_mul(out=msq2, in0=grp[:, 0:1], in1=grp[:, 0:1])
                nc.vector.tensor_tensor(
                    out=grp[:, 1:2], in0=grp[:, 1:2], in1=msq2, op=OP.subtract
                )
                nc.scalar.activation(
                    out=grp[:, 1:2], in_=grp[:, 1:2], func=AF.Sqrt,
                    bias=eps_col[:GROUPS],
                )
                nc.vector.reciprocal(out=grp[:, 1:2], in_=grp[:, 1:2])

                # per-channel affine: gn(x) = A*x + Cc
                AC = persist.tile([P, 2, 2], F32, tag="ac")
                gnq = stg.tile([P, 2, NPIX], BF16, tag="gnq")
                for t in range(2):
                    ps = ps1.tile([P, 1024], F32, tag="p")
                    nc.tensor.matmul(
                        ps[:, 0:2], gbc[:, t * P : (t + 1) * P], grp,
                        start=True, stop=True,
                    )
                    nc.vector.tensor_mul(
                        out=AC[:, t, 0:1], in0=gnw[:, t : t + 1], in1=ps[:, 1:2]
                    )
                    mt_ = tmp.tile([P, 1], F32, tag="msq")
                    nc.vector.tensor_mul(out=mt_, in0=ps[:, 0:1], in1=AC[:, t, 0:1])
                    nc.vector.tensor_tensor(
                        out=AC[:, t, 1:2], in0=gnb[:, t : t + 1], in1=mt_,
                        op=OP.subtract,
                    )
                    nc.gpsimd.tensor_scalar(
                        out=gnq[:, t], in0=xq_sb[:, t],
                        scalar1=AC[:, t, 0:1], scalar2=AC[:, t, 1:2],
                        op0=OP.mult, op1=OP.add,
                    )

                # fold bo into the residual: xq_sb += bo (per partition)
                for t in range(2):
                    nc.gpsimd.tensor_scalar(
                        out=xq_sb[:, t], in0=xq_sb[:, t],
                        scalar1=bop[:, t : t + 1], scalar2=None, op0=OP.add,
                    )

                for g in range(4):
                    proj_pair(g, wq_bf, gnq, QK_CHUNKS, qz, bqp, False)

            # ---------------- attention ----------------
            with (
                tc.tile_pool(name="attn", bufs=6) as atp,
                tc.tile_pool(name="rcb_p", bufs=2) as rcb_p,
                tc.tile_pool(name="rdram", bufs=2, space="DRAM") as rdram,
                tc.tile_pool(name="psqk", bufs=3, space="PSUM") as psqk,
                tc.tile_pool(name="psav", bufs=2, space="PSUM") as psav,
            ):
                # bridge the proj->attention transition so HAM stays warm
                warm = psqk.tile([P, 1024], F32, tag="qk")
                for i in range(6):
                    nc.tensor.matmul(
                        warm[:, 0:512],
                        kpair[:, 0, 0:P],
                        qz[:, 0, 0, 0:512],
                        start=True, stop=True,
                    )
                for g in range(4):
                    for qci, (qo, qw) in enumerate(Q_CHUNKS):
                        pav = psav.tile([P, 512], F32, tag="av")
                        # batch kv-tiles per psum tile so small q-chunks get
                        # one big exp instead of many tiny latency-bound ones
                        GRP = max(1, 512 // qw)
                        groups = [
                            list(range(s, min(s + GRP, KT)))
                            for s in range(0, KT, GRP)
                        ]
                        for gi, grp_kts in enumerate(groups):
                            qk = psqk.tile([P, 1024], F32, tag="qk")
                            for j, kt in enumerate(grp_kts):
                                if (qw == 512 and kt % 3 != 0) or (
                                    qw == 128 and j % 4 != 0
                                ):
                                    # row-tiled concurrent pair: half the
                                    # streamed columns; the surrounding
                                    # full-array matmuls keep HAM warm
                                    nc.tensor.matmul(
                                        qk[:, j * qw : j * qw + qw],
                                        kpair[0:D, g, kt * P : (kt + 1) * P],
                                        qz[0:D, 0, g, qo : qo + qw],
                                        start=True, stop=True,
                                    )
                                    nc.tensor.matmul(
                                        qk[:, 512 + j * qw : 512 + j * qw + qw],
                                        kpair[64 : 64 + D, g, kt * P : (kt + 1) * P],
                                        qz[64 : 64 + D, 1, g, qo : qo + qw],
                                        start=True, stop=True,
                                    )
                                    continue
                                # full-K=128 matmuls vs the whole kpair
                                # block; zero-masked q selects the head.
                                nc.tensor.matmul(
                                    qk[:, j * qw : (j + 1) * qw],
                                    kpair[:, g, kt * P : (kt + 1) * P],
                                    qz[:, 0, g, qo : qo + qw],
                                    start=True, stop=True,
                                )
                                nc.tensor.matmul(
                                    qk[:, 512 + j * qw : 512 + (j + 1) * qw],
                                    kpair[:, g, kt * P : (kt + 1) * P],
                                    qz[:, 1, g, qo : qo + qw],
                                    start=True, stop=True,
                                )
                            at = atp.tile([P, 1024], BF16, tag="at")
                            gw = len(grp_kts) * qw
                            qk_v = qk.rearrange("p (b c) -> p b c", b=2)[:, :, 0:gw]
                            at_v = at.rearrange("p (b c) -> p b c", b=2)[:, :, 0:gw]
                            if ACT_KT[gi]:
                                nc.scalar.activation(
                                    out=at_v, in_=qk_v, func=AF.Exp, scale=SCALE,
                                    bias=zeros_col,
                                )
                            else:
                                nc.vector.tensor_scalar(
                                    out=at_v.bitcast(U16), in0=qk_v,
                                    scalar1=A_EXP, scalar2=B_EXP,
                                    op0=OP.mult, op1=OP.add,
                                )
                            # AV accumulate; col-tiled pair shares one bank
                            for j, kt in enumerate(grp_kts):
                                nc.tensor.matmul(
                                    pav[0:VW, 0:qw],
                                    vT[:, kt, 2 * g],
                                    at[:, j * qw : (j + 1) * qw],
                                    start=(kt == 0), stop=(kt == KT - 1),
                                )
                                nc.tensor.matmul(
                                    pav[VW:P, 0:qw],
                                    vT[:, kt, 2 * g + 1],
                                    at[:, 512 + j * qw : 512 + (j + 1) * qw],
                                    start=(kt == 0), stop=(kt == KT - 1),
                                )
                        # drain unnormalized o (+rowsum rows 63/127) to o_pad
                        if (g * 3 + qci) % 2 == 0:
                            nc.scalar.copy(
                                out=o_pad[:, g, qo : qo + qw], in_=pav[:, 0:qw]
                            )
                        else:
                            nc.vector.tensor_copy(
                                out=o_pad[:, g, qo : qo + qw], in_=pav[:, 0:qw]
                            )

                    # normalize pair g: 1/rowsum bit-trick + broadcast + mult
                    # (rowsums at partitions 0 / 64 - 32-aligned for the DVE)
                    rc = tmp.tile([P, QH], BF16, tag="rc")
                    for row in (0, VW):
                        nc.gpsimd.tensor_scalar(
                            out=rc[row : row + 1].bitcast(U16),
                            in0=o_pad[row : row + 1, g, :].bitcast(U16),
                            scalar1=-1.0, scalar2=MAGIC16,
                            op0=OP.mult, op1=OP.add,
                        )
                    rdr = rdram.tile([2, QH], BF16, tag="rdr")
                    for i, row in enumerate((0, VW)):
                        nc.sync.dma_start(
                            out=rdr[i : i + 1], in_=rc[row : row + 1]
                        )
                    rcb = rcb_p.tile([P, QH], BF16, tag="rcb")
                    for h in range(2):
                        rowap = rdr[h : h + 1, :]
                        src = bass.AP(
                            tensor=rowap.tensor, offset=rowap.offset,
                            ap=[[0, VW]] + list(rowap.ap[1:]),
                        )
                        nc.sync.dma_start(out=rcb[VW * h : VW * (h + 1), :], in_=src)
                    nc.gpsimd.tensor_tensor(
                        out=o_pad[:, g, :], in0=o_pad[:, g, :], in1=rcb,
                        op=OP.mult,
                    )
                    # warm bridge: keep the PE clock gate fed across the
                    # pair boundary
                    wtile = psqk.tile([P, 1024], F32, tag="qk")
                    for _ in range(4):
                        nc.tensor.matmul(
                            wtile[:, 0:512],
                            kpair[:, g, 0:P],
                            qz[:, 0, g, 0:512],
                            start=True, stop=True,
                        )

                # ---------------- output projection + residual ----------------
                for mt in range(2):
                    for (qo, qw) in Q_CHUNKS:
                        ps = psqk.tile([P, 1024], F32, tag="qk")
                        for kp in range(4):
                            nc.tensor.matmul(
                                ps[:, 0:qw],
                                wo_bf[:, kp, mt * P : (mt + 1) * P],
                                o_pad[:, kp, qo : qo + qw],
                                start=(kp == 0), stop=(kp == 3),
                            )
                        osb = tmp.tile([P, 512], F32, tag="osb")
                        nc.vector.tensor_tensor(
                            out=osb[:, 0:qw], in0=ps[:, 0:qw],
                            in1=xq_sb[:, mt, qo : qo + qw], op=OP.add,
                        )
                        nc.sync.dma_start(
                            out=out_d[mt * P : (mt + 1) * P, qo : qo + qw],
                            in_=osb[:, 0:qw],
                        )
    nc.finalize()
    return nc


_CACHE = {}


def _get_nc():
    if "nc" not in _CACHE:
        _CACHE["nc"] = _build()
    return _CACHE["nc"]


def _host_consts():
    if "consts" in _CACHE:
        return _CACHE["consts"]
    gsum = np.zeros((P, 2, GROUPS), np.float32)
    for t in range(2):
        for p in range(P):
            gsum[p, t, 16 * t + p // 8] = 1.0 / 8.0
    gbc = np.zeros((GROUPS, C), np.float32)
    for c in range(C):
        gbc[c // 8, c] = 1.0
    _CACHE["consts"] = (gsum, gbc)
    return _CACHE["consts"]


def _pair_wo(woT):
    # [384, 256] -> [512, 256]; head h rows at 128*(h//2) + 64*(h%2) + 1
    # (+1: row 0 of each half holds the rowsum from the ones-column of V)
    out = np.zeros((4 * P, C), np.float32)
    for g in range(4):
        for half in range(2):
            r0 = P * g + 64 * half + 1
            out[r0 : r0 + D] = woT[96 * g + D * half : 96 * g + D * half + D]
    return out


def _pair_wT(wT):
    # [256, 384] -> [256, 512]; head h cols at 128*(h//2) + 64*(h%2)
    out = np.zeros((C, 4 * P), np.float32)
    for g in range(4):
        for half in range(2):
            out[:, P * g + 64 * half : P * g + 64 * half + D] = wT[
                :, 96 * g + D * half : 96 * g + D * half + D
            ]
    return out


def _pair_bias(b):
    out = np.zeros((P, 4), np.float32)
    for g in range(4):
        out[0:48, g] = b[96 * g : 96 * g + 48]
        out[64:112, g] = b[96 * g + 48 : 96 * g + 96]
    return out


def _split_bias(b):
    # [2k*128] -> [128, 2k] partition-major
    n = b.shape[0] // P
    return np.ascontiguousarray(b.reshape(n, P).T)


def run(inputs, **kwargs):
    q_feat = np.asarray(inputs["q_feat"], np.float32).reshape(B, C, NPIX)
    kv_feat = np.asarray(inputs["kv_feat"], np.float32).reshape(B, C, NPIX)
    wqT = _pair_wT(np.ascontiguousarray(np.asarray(inputs["wq"], np.float32).T))
    wkT = _pair_wT(np.ascontiguousarray(np.asarray(inputs["wk"], np.float32).T))
    wvT = np.ascontiguousarray(np.asarray(inputs["wv"], np.float32).T)
    woT = _pair_wo(np.ascontiguousarray(np.asarray(inputs["wo"], np.float32).T))
    bqp = _pair_bias(np.asarray(inputs["bq"], np.float32))
    # fold bv through wo (softmax rows sum to 1) and bk away (shift inv.)
    bo_eff = np.asarray(inputs["bo"], np.float32) + (
        np.asarray(inputs["wo"], np.float32) @ np.asarray(inputs["bv"], np.float32)
    )
    bop = _split_bias(bo_eff)
    gnwp = _split_bias(np.asarray(inputs["gn_w"], np.float32))
    gnbp = _split_bias(np.asarray(inputs["gn_b"], np.float32))
    gsum, gbc = _host_consts()

    in_maps = []
    for b in range(B):
        for j in range(2):
            # roll so this core's query pixels land at columns 0..QH-1;
            # GroupNorm stats are permutation-invariant, kv side unaffected
            in_maps.append(
                {
                    "xq": np.ascontiguousarray(np.roll(q_feat[b], -QH * j, axis=1)),
                    "xkv": np.ascontiguousarray(kv_feat[b]),
                    "wqT": wqT,
                    "wkT": wkT,
                    "wvT": wvT,
                    "woT": woT,
                    "bqp": bqp,
                    "bop": bop,
                    "gnwp": gnwp,
                    "gnbp": gnbp,
                    "gsum": gsum,
                    "gbc": gbc,
                }
            )

    res = run_bass_kernel_spmd(_get_nc(), in_maps, core_ids=list(range(8)), **kwargs)

    out = np.empty((B, C, NPIX), np.float32)
    for i, r in enumerate(res.results):
        b, j = divmod(i, 2)
        out[b, :, QH * j : QH * (j + 1)] = r["out"]
    return out.reshape(B, C, 48, 48), res


def kernel(**inputs):
    out, _ = run(inputs)
    return out
